# revision 4
# baseline (speedup 1.0000x reference)
"""Trainium2 Bass kernel for nn_CentroidDiscoverBlock (vq_codebook).

Shapes (hardcoded): STFeature [4, 8, 4096, 256] f32, centroidsTemp [4, 64, 256] f32.

Strategy
--------
All the heavy compute in this block reduces to, per batch b:
    scores[r, l] = STF[b, r, :] . Qk[b, l, :]   (Qk = (centroids@qc_w.T+qc_b)@nk_w)
    assign[r]    = argmax_l scores[r, l]        (as one-hot via score >= rowmax)
    sums[b, l]   = sum of raw STF rows assigned to cluster l ; counts[b, l]
because the K/V projections commute with the cross-attention contraction and
the cluster scatter-sum respectively:
    Q.(nk_w@x+nk_b) = (nk_w.T@Q).x + Q.nk_b   and
    sum_r nv(x_r) = nv_w @ (sum_r x_r) + count*nv_b.
This removes both [B,T,N,C]x[C,C] projections (2x17 GFLOP) entirely.

Sharding: core = 2*b + half; each of the 8 cores handles one (b, half of T*N)
shard of 16384 rows. The host pre-packs the shard in fp8 twice (the score /
scatter operands tolerate fp8: the cluster-mean path is divided by counts^2+1
so the end-to-end deviation stays ~1.5e-5 relative):
  * stft: C-on-partition layout, the stationary operand of the score matmuls
    (a C-contraction needs C on partitions; FWL hides the 128-col weight
    loads behind the 64-col moving qkt),
  * stf4: rows-on-partition layout, partition-major in DRAM so each DMA
    descriptor is one large contiguous per-partition run, with a fused ones
    column so one PE matmul accumulates sums AND counts.

Per 2048-row group the pipeline is spread over four engines:
  PE    : 32 score matmuls (stationary stft slice, moving qkt) -> PSUM
  ACT   : PSUM -> SBUF copy to bf16, transposed per-partition to l-major
          [128, L, rows] so the one-hot compare below hits the DVE 2x mode
  DVE   : reduce_max over l (strided view), then is_ge against the row max
          broadcast along l -- all operands 2-byte packed => 2x rate
  PE    : 16 scatter matmuls (stationary one-hot, moving stf4) accumulating
          [64, 257] sums|counts in PSUM (two banks, alternating)
A handful of warm-up matmuls on scratch SBUF keep the PE HAM clock-gate warm
while the first DMA pieces stream in.

The [64, 257] per-core partials are summed pairwise on host and the tiny
[4, 64, 256] epilogue (cluster means, MHA over 64 centroids, BatchNorm over
(B,L), FFN -- ~0.1% of the FLOPs) runs in fp32 numpy.
"""

from contextlib import ExitStack

import ml_dtypes
import numpy as np

import concourse.bass as bass
import concourse.mybir as mybir
import concourse.tile as tile
from concourse.bass_utils import run_bass_kernel_spmd

F32 = mybir.dt.float32
BF16 = mybir.dt.bfloat16
NP_BF16 = ml_dtypes.bfloat16
FP8 = mybir.dt.float8e4
NP_FP8 = ml_dtypes.float8_e4m3
P = 128
B, T, N = 4, 8, 4096
C = 256
L = 64
R = 4  # rows per partition in the natural packing (512-row chunks)
N_HEADS = 4
BN_EPS = 1e-5
ROWS_PER_CORE = T * N // 2  # 16384
N_CHUNKS = ROWS_PER_CORE // (P * R)  # 32
SC = 4  # chunks per pipeline group
G = N_CHUNKS // SC  # 8 groups
SPANS = [1, 1, 2, 4, 6, 6, 6, 6]  # DMA piece sizes in chunks (sum = 32)

# one-hot dtype: bf16 keeps the DVE is_ge in its 2x (all-16-bit) mode; the
# scatter matmul then runs with bf16 stationary x fp8 moving.
ONEHOT_DT = BF16

N_WARM = 8  # PE warm-up matmuls (~3.4us at cold clock = one HAM window)

SYNC_WAIT_LIMIT = 1

# test.py hooks: set PROFILE=True before calling kernel() to capture an NTFF
# trace; exec time lands in LAST_EXEC_TIME_NS.
PROFILE = False
LAST_EXEC_TIME_NS = None
LAST_RESULTS = None


def _split_sync_waits(nc: bass.Bass, limit: int = SYNC_WAIT_LIMIT):
    # This walrus build rejects instructions carrying more than `limit` sync
    # waits ("Too many sync wait commands" in CoreV3 codegen setupSyncWait).
    # Hoist excess waits onto standalone EventSemaphore instructions placed
    # immediately before the owner on the same engine (engine streams are
    # in-order, so the conditions still hold when the owner issues).
    n = 0
    for fn in nc.m.functions:
        for bb in fn.blocks:
            insts = bb.instructions
            if not any(
                i.sync_info is not None and len(i.sync_info.on_wait) > limit
                for i in insts
            ):
                continue
            out = []
            for inst in insts:
                si = inst.sync_info
                if si is not None and len(si.on_wait) > limit:
                    waits = list(si.on_wait)
                    excess, keep = waits[:-limit], waits[-limit:]
                    for j in range(0, len(excess), limit):
                        ev = mybir.InstEventSemaphore(
                            name=f"{inst.name}-sw{n}", ins=[], outs=[]
                        )
                        n += 1
                        ev.engine = inst.engine
                        ev.sync_info = mybir.SyncInfo(
                            on_wait=excess[j : j + limit], on_update=[]
                        )
                        out.append(ev)
                    inst.sync_info = mybir.SyncInfo(
                        on_wait=keep, on_update=list(si.on_update)
                    )
                out.append(inst)
            bb.instructions = out


def _build(n_chunks: int, with_qb: bool, split: bool = True) -> bass.Bass:
    rows = n_chunks * P * R
    nc = bass.Bass("TRN2", target_bir_lowering=False, debug=False)

    # [2, 128, rows] fp8; half h holds C-dims [128h, 128h+128), columns
    # ordered (chunk, r, p) <-> row chunk*512 + 4p + r
    stft_d = nc.dram_tensor("stft", [2, P, rows], FP8, kind="ExternalInput")
    # [128, n_chunks, 4*257] fp8, partition-major; (p, chunk, r, c) <-> row
    # chunk*512 + 4p + r, c==256 is the ones column
    stf4_d = nc.dram_tensor("stf4", [P, n_chunks, R * (C + 1)], FP8,
                            kind="ExternalInput")
    qkt_d = nc.dram_tensor("qkt", [2, P, L], FP8, kind="ExternalInput")
    qb_d = None
    if with_qb:
        qb_d = nc.dram_tensor("qb_bc", [P, L], F32, kind="ExternalInput")
    out_d = nc.dram_tensor("out_sums", [L, C + 1], F32, kind="ExternalOutput")

    with tile.TileContext(nc) as tc, ExitStack() as ctx:
        consts = ctx.enter_context(tc.tile_pool(name="consts", bufs=1))
        scb_pool = ctx.enter_context(tc.tile_pool(name="scb", bufs=2))
        oh_pool = ctx.enter_context(tc.tile_pool(name="oh", bufs=2))
        rm_pool = ctx.enter_context(tc.tile_pool(name="rm", bufs=2))
        psum_s = ctx.enter_context(tc.tile_pool(name="psum_s", bufs=2, space="PSUM"))
        psum_acc = ctx.enter_context(tc.tile_pool(name="psum_acc", bufs=1, space="PSUM"))
        psum_w = ctx.enter_context(tc.tile_pool(name="psum_w", bufs=1, space="PSUM"))

        # ---- input DMA schedule (SP ring, FIFO): small first pieces so the
        # score matmuls can start as soon as chunk 0 lands, stft/stf4
        # interleaved so both streams advance together.
        qkt_t = consts.tile([P, 2, L], FP8)
        nc.sync.dma_start(qkt_t[:, 0, :], qkt_d[0])
        nc.sync.dma_start(qkt_t[:, 1, :], qkt_d[1])
        qb_t = None
        if with_qb:
            qb_t = consts.tile([P, L], F32)
            nc.sync.dma_start(qb_t[:], qb_d[:])

        stft0 = consts.tile([P, n_chunks, R, P], FP8, tag="stft0")
        stft1 = consts.tile([P, n_chunks, R, P], FP8, tag="stft1")
        stf4 = consts.tile([P, n_chunks, R * (C + 1)], FP8, tag="stf4")
        bounds = [0]
        for s in SPANS:
            bounds.append(bounds[-1] + s)
        assert bounds[-1] == n_chunks
        for lo, hi in zip(bounds[:-1], bounds[1:]):
            sl = slice(lo * R * P, hi * R * P)
            nc.sync.dma_start(stft0[:, lo:hi, :, :], stft_d[0][:, sl])
            nc.sync.dma_start(stft1[:, lo:hi, :, :], stft_d[1][:, sl])
            nc.sync.dma_start(stf4[:, lo:hi, :], stf4_d[:, lo:hi, :])

        # ---- PE warm-up: keep the HAM activity window busy while the first
        # DMA pieces stream in, so the real matmuls start at the warm clock.
        scratch = consts.tile([P, 512], FP8, tag="warm")
        nc.gpsimd.memset(scratch[:], 0)
        warm_ps = psum_w.tile([P, 512], F32)
        for _ in range(N_WARM):
            nc.tensor.matmul(
                warm_ps[:], scratch[:, :P], scratch[:],
                start=True, stop=True, skip_group_check=True,
            )

        # two PSUM accumulators (alternating per scatter matmul) so
        # consecutive accumulates never target the same bank back-to-back
        acc = [
            psum_acc.tile([L, C + 1], F32, tag="acc0", name="acc0"),
            psum_acc.tile([L, C + 1], F32, tag="acc1", name="acc1"),
        ]
        n_scatter = G * SC * R

        def scores(g):
            ps = psum_s.tile([P, SC * R, L], F32)
            for i in range(SC):
                chunk = g * SC + i
                for r in range(R):
                    nc.tensor.matmul(
                        ps[:, i * R + r, :], stft0[:, chunk, r, :],
                        qkt_t[:, 0, :], start=True, stop=False,
                    )
                    nc.tensor.matmul(
                        ps[:, i * R + r, :], stft1[:, chunk, r, :],
                        qkt_t[:, 1, :], start=False, stop=True,
                    )
            return ps

        def onehot_of(g, ps):
            # ACT: PSUM fp32 [128, 16, 64] -> SBUF bf16 l-major [128, 64, 16]
            scb = scb_pool.tile([P, L, SC * R], BF16, tag="scb")
            scb_rl = scb[:].rearrange("p l r -> p r l")
            if with_qb:
                nc.scalar.activation(
                    scb_rl, ps[:], mybir.ActivationFunctionType.Copy,
                    bias=qb_t[:],  # unreachable for zero nk_b; kept for safety
                )
            else:
                nc.scalar.copy(scb_rl, ps[:])
            # DVE: row max over l (strided innermost view), then one-hot
            # compare with the max broadcast along l. All operands bf16 and
            # packed in the last dim => DVE 2x mode.
            rowmax = rm_pool.tile([P, SC * R], BF16, tag="rmax")
            nc.vector.reduce_max(rowmax[:], scb_rl, axis=mybir.AxisListType.X)
            onehot = oh_pool.tile([P, L, SC * R], ONEHOT_DT, tag="oh")
            nc.vector.tensor_tensor(
                out=onehot[:], in0=scb[:],
                in1=rowmax[:].unsqueeze(1).to_broadcast([P, L, SC * R]),
                op=mybir.AluOpType.is_ge,
            )
            return onehot

        def scatter(g, onehot):
            for i in range(SC):
                chunk = g * SC + i
                for r in range(R):
                    j = i * R + r
                    gidx = g * SC * R + j
                    nc.tensor.matmul(
                        acc[gidx % 2][:], onehot[:, :, j],
                        stf4[:, chunk, r * (C + 1) : (r + 1) * (C + 1)],
                        start=(gidx < 2), stop=(gidx >= n_scatter - 2),
                        skip_group_check=True,
                    )

        ps_g = scores(0)
        oh_g = None
        for g in range(G):
            ps_next = scores(g + 1) if g + 1 < G else None
            oh_g = onehot_of(g, ps_g)
            scatter(g, oh_g)
            ps_g = ps_next

        sums_tmp = consts.tile([L, C + 1], F32)
        nc.scalar.copy(sums_tmp[:], acc[0][:])
        sums_sb = consts.tile([L, C + 1], F32)
        nc.vector.tensor_tensor(
            out=sums_sb[:], in0=sums_tmp[:], in1=acc[1][:],
            op=mybir.AluOpType.add,
        )
        nc.sync.dma_start(out_d[:], sums_sb[:])

    if split:
        _split_sync_waits(nc)
    return nc


def _pack_shard(rows_f32: np.ndarray):
    """rows_f32: [rows, 256] f32 -> (stft [2,128,rows] fp8, stf4 [128,nc,1028] fp8)."""
    rows = rows_f32.shape[0]
    n_chunks = rows // (P * R)
    a = rows_f32.reshape(n_chunks, P, R, C)
    a8 = a.astype(NP_FP8)
    stf4 = np.ascontiguousarray(
        np.concatenate([a8, np.ones((n_chunks, P, R, 1), NP_FP8)], axis=-1)
        .transpose(1, 0, 2, 3)
    ).reshape(P, n_chunks, R * (C + 1))
    stft = np.ascontiguousarray(a8.transpose(3, 0, 2, 1)).reshape(2, P, rows)
    return stft, stf4


def _softmax(x, axis):
    m = np.max(x, axis=axis, keepdims=True)
    e = np.exp(x - m)
    return e / np.sum(e, axis=axis, keepdims=True)


def kernel(STFeature, centroidsTemp, qc_w, qc_b, nk_w, nk_b, nv_w, nv_b,
           al_w, al_b, mq_w, mq_b, mk_w, mk_b, mv_w, mv_b, mo_w, mo_b,
           bn_gamma, bn_beta, alpha, bias, ff1_w, ff1_b, ff2_w, ff2_b):
    global LAST_EXEC_TIME_NS, LAST_RESULTS
    f = np.float32
    STFeature = np.asarray(STFeature, f)
    centroidsTemp = np.asarray(centroidsTemp, f)

    # host-side prep (tiny): fold the node-key projection into the query side
    q_cent = centroidsTemp @ np.asarray(qc_w, f).T + np.asarray(qc_b, f)  # [B,L,C]
    qk = q_cent @ np.asarray(nk_w, f)                                     # [B,L,C]
    qb = q_cent @ np.asarray(nk_b, f)                                     # [B,L]
    with_qb = bool(np.any(qb != 0.0))

    in_maps = []
    flat = STFeature.reshape(B, T * N, C)
    for core in range(8):
        b, half = divmod(core, 2)
        stft, stf4 = _pack_shard(
            flat[b, half * ROWS_PER_CORE : (half + 1) * ROWS_PER_CORE]
        )
        m = {
            "stft": stft,
            "stf4": stf4,
            "qkt": np.ascontiguousarray(qk[b].T.reshape(2, P, L)).astype(NP_FP8),
        }
        if with_qb:
            m["qb_bc"] = np.ascontiguousarray(np.tile(qb[b][None, :], (P, 1)))
        in_maps.append(m)

    # the axon-proxied device occasionally reports a transient
    # NRT_EXEC_UNIT_UNRECOVERABLE; a fresh build+run attempt recovers it
    last_exc = None
    for attempt in range(3):
        try:
            nc = _build(N_CHUNKS, with_qb)
            res = run_bass_kernel_spmd(
                nc, in_maps, core_ids=list(range(8)), trace=bool(PROFILE)
            )
            break
        except Exception as e:
            last_exc = e
            import time as _time
            _time.sleep(15)
    else:
        raise last_exc
    LAST_EXEC_TIME_NS = res.exec_time_ns
    LAST_RESULTS = res

    sums = np.zeros((B, L, C), f)
    counts = np.zeros((B, L), f)
    for b in range(B):
        p0 = res.results[2 * b]["out_sums"]
        p1 = res.results[2 * b + 1]["out_sums"]
        sums[b] = p0[:, :C] + p1[:, :C]
        counts[b] = p0[:, C] + p1[:, C]

    # tiny epilogue on host, fp32 (mirrors the reference math)
    sums_v = sums @ np.asarray(nv_w, f).T + counts[..., None] * np.asarray(nv_b, f)
    cluster = sums_v / (counts**2 + 1.0)[..., None]
    cent = centroidsTemp + cluster @ np.asarray(al_w, f).T + np.asarray(al_b, f)

    D = cent.shape[-1]
    hd = D // N_HEADS
    q = (cent @ np.asarray(mq_w, f).T + np.asarray(mq_b, f)).reshape(B, L, N_HEADS, hd)
    k = (cent @ np.asarray(mk_w, f).T + np.asarray(mk_b, f)).reshape(B, L, N_HEADS, hd)
    v = (cent @ np.asarray(mv_w, f).T + np.asarray(mv_b, f)).reshape(B, L, N_HEADS, hd)
    logits = np.einsum("bqhd,bkhd->bhqk", q, k) / np.sqrt(f(hd))
    attn = _softmax(logits, axis=-1)
    attn_out = np.einsum("bhqk,bkhd->bqhd", attn, v).reshape(B, L, D)
    attn_out = attn_out @ np.asarray(mo_w, f).T + np.asarray(mo_b, f)

    z2 = cent + attn_out
    mean = z2.mean(axis=(0, 1))
    var = ((z2 - mean) ** 2).mean(axis=(0, 1))
    zn = (z2 - mean) / np.sqrt(var + f(BN_EPS))
    zn = np.asarray(bn_gamma, f) * zn + np.asarray(bn_beta, f)
    zn = np.asarray(alpha, f) * zn + np.asarray(bias, f)

    h = np.maximum(zn @ np.asarray(ff1_w, f).T + np.asarray(ff1_b, f), 0.0)
    out = h @ np.asarray(ff2_w, f).T + np.asarray(ff2_b, f)
    return out.astype(np.float32)


# revision 7
# speedup vs baseline: 1.5676x; 1.5676x over previous
"""Trainium2 Bass kernel for nn_CentroidDiscoverBlock (vq_codebook).

Shapes (hardcoded): STFeature [4, 8, 4096, 256] f32, centroidsTemp [4, 64, 256] f32.

Strategy
--------
All the heavy compute in this block reduces to, per batch b:
    scores[r, l] = STF[b, r, :] . Qk[b, l, :]   (Qk = (centroids@qc_w.T+qc_b)@nk_w)
    assign[r]    = argmax_l scores[r, l]        (as one-hot via score >= rowmax)
    sums[b, l]   = sum of raw STF rows assigned to cluster l ; counts[b, l]
because the K/V projections commute with the cross-attention contraction and
the cluster scatter-sum respectively:
    Q.(nk_w@x+nk_b) = (nk_w.T@Q).x + Q.nk_b   and
    sum_r nv(x_r) = nv_w @ (sum_r x_r) + count*nv_b.
This removes both [B,T,N,C]x[C,C] projections (2x17 GFLOP) entirely.

Sharding: core = 2*b + half; each of the 8 cores handles one (b, half of T*N)
shard of 16384 rows. The host pre-packs the shard in fp8 twice (the score /
scatter operands tolerate fp8: the cluster-mean path is divided by counts^2+1
so the end-to-end deviation stays ~1.5e-5 relative):
  * stft: C-on-partition layout, the stationary operand of the score matmuls
    (a C-contraction needs C on partitions; FWL hides the 128-col weight
    loads behind the 64-col moving qkt),
  * stf4: rows-on-partition layout, partition-major in DRAM so each DMA
    descriptor is one large contiguous per-partition run, with a fused ones
    column so one PE matmul accumulates sums AND counts.

Per 2048-row group the pipeline is spread over four engines:
  PE    : 32 score matmuls (stationary stft slice, moving qkt) -> PSUM
  ACT   : PSUM -> SBUF copy to bf16, transposed per-partition to l-major
          [128, L, rows] so the one-hot compare below hits the DVE 2x mode
  DVE   : reduce_max over l (strided view), then is_ge against the row max
          broadcast along l -- all operands 2-byte packed => 2x rate
  PE    : 16 scatter matmuls (stationary one-hot, moving stf4) accumulating
          [64, 257] sums|counts in PSUM (two banks, alternating)
A handful of warm-up matmuls on scratch SBUF keep the PE HAM clock-gate warm
while the first DMA pieces stream in.

The [64, 257] per-core partials are summed pairwise on host and the tiny
[4, 64, 256] epilogue (cluster means, MHA over 64 centroids, BatchNorm over
(B,L), FFN -- ~0.1% of the FLOPs) runs in fp32 numpy.
"""

from contextlib import ExitStack

import ml_dtypes
import numpy as np

import concourse.bass as bass
import concourse.mybir as mybir
import concourse.tile as tile
from concourse.bass_utils import run_bass_kernel_spmd

F32 = mybir.dt.float32
BF16 = mybir.dt.bfloat16
NP_BF16 = ml_dtypes.bfloat16
FP8 = mybir.dt.float8e4
NP_FP8 = ml_dtypes.float8_e4m3
P = 128
B, T, N = 4, 8, 4096
C = 256
L = 64
R = 4  # rows per partition in the natural packing (512-row chunks)
N_HEADS = 4
BN_EPS = 1e-5
ROWS_PER_CORE = T * N // 2  # 16384
N_CHUNKS = ROWS_PER_CORE // (P * R)  # 32
SC = 4  # chunks per pipeline group
G = N_CHUNKS // SC  # 8 groups
SPANS = [2, 2, 4, 4, 6, 7, 7]  # DMA piece sizes in chunks (sum = 32)

N_WARM = 8  # PE warm-up matmuls (~3.4us at cold clock = one HAM window)

SYNC_WAIT_LIMIT = 1

# test.py hooks: set PROFILE=True before calling kernel() to capture an NTFF
# trace; exec time lands in LAST_EXEC_TIME_NS.
PROFILE = False
LAST_EXEC_TIME_NS = None
LAST_RESULTS = None


def _split_sync_waits(nc: bass.Bass, limit: int = SYNC_WAIT_LIMIT):
    # This walrus build rejects instructions carrying more than `limit` sync
    # waits ("Too many sync wait commands" in CoreV3 codegen setupSyncWait).
    # Hoist excess waits onto standalone EventSemaphore instructions placed
    # immediately before the owner on the same engine (engine streams are
    # in-order, so the conditions still hold when the owner issues).
    n = 0
    for fn in nc.m.functions:
        for bb in fn.blocks:
            insts = bb.instructions
            if not any(
                i.sync_info is not None and len(i.sync_info.on_wait) > limit
                for i in insts
            ):
                continue
            out = []
            for inst in insts:
                si = inst.sync_info
                if si is not None and len(si.on_wait) > limit:
                    waits = list(si.on_wait)
                    excess, keep = waits[:-limit], waits[-limit:]
                    for j in range(0, len(excess), limit):
                        ev = mybir.InstEventSemaphore(
                            name=f"{inst.name}-sw{n}", ins=[], outs=[]
                        )
                        n += 1
                        ev.engine = inst.engine
                        ev.sync_info = mybir.SyncInfo(
                            on_wait=excess[j : j + limit], on_update=[]
                        )
                        out.append(ev)
                    inst.sync_info = mybir.SyncInfo(
                        on_wait=keep, on_update=list(si.on_update)
                    )
                out.append(inst)
            bb.instructions = out


def _build(n_chunks: int, with_qb: bool, split: bool = True) -> bass.Bass:
    rows = n_chunks * P * R
    nc = bass.Bass("TRN2", target_bir_lowering=False, debug=False)

    # [2, 128, rows] fp8; half h holds C-dims [128h, 128h+128), columns
    # ordered (chunk, r, p) <-> row chunk*512 + 4p + r
    stft_d = nc.dram_tensor("stft", [2, P, rows], FP8, kind="ExternalInput")
    # [128, n_chunks, 4*257] fp8, partition-major; (p, chunk, r, c) <-> row
    # chunk*512 + 4p + r, c==256 is the ones column
    stf4_d = nc.dram_tensor("stf4", [P, n_chunks, R * (C + 1)], FP8,
                            kind="ExternalInput")
    qkt_d = nc.dram_tensor("qkt", [2, P, L], FP8, kind="ExternalInput")
    qb_d = None
    if with_qb:
        qb_d = nc.dram_tensor("qb_bc", [P, L], F32, kind="ExternalInput")
    out_d = nc.dram_tensor("out_sums", [L, C + 1], F32, kind="ExternalOutput")

    with tile.TileContext(nc) as tc, ExitStack() as ctx:
        consts = ctx.enter_context(tc.tile_pool(name="consts", bufs=1))
        scb_pool = ctx.enter_context(tc.tile_pool(name="scb", bufs=2))
        oh_pool = ctx.enter_context(tc.tile_pool(name="oh", bufs=2))
        rm_pool = ctx.enter_context(tc.tile_pool(name="rm", bufs=2))
        psum_s = ctx.enter_context(tc.tile_pool(name="psum_s", bufs=2, space="PSUM"))
        psum_acc = ctx.enter_context(tc.tile_pool(name="psum_acc", bufs=1, space="PSUM"))
        psum_w = ctx.enter_context(tc.tile_pool(name="psum_w", bufs=1, space="PSUM"))

        # ---- input DMA schedule: small first pieces so the score matmuls can
        # start as soon as chunk 0 lands. The stft (scores) stream goes on the
        # SP hardware-DGE ring, the stf4 (scatter) stream on the ACT ring, so
        # descriptor generation and draining run in parallel.
        qkt_t = consts.tile([P, 2, L], FP8)
        nc.sync.dma_start(qkt_t[:, 0, :], qkt_d[0])
        nc.sync.dma_start(qkt_t[:, 1, :], qkt_d[1])
        qb_t = None
        if with_qb:
            qb_t = consts.tile([P, L], F32)
            nc.sync.dma_start(qb_t[:], qb_d[:])

        stft0 = consts.tile([P, n_chunks, R, P], FP8, tag="stft0")
        stft1 = consts.tile([P, n_chunks, R, P], FP8, tag="stft1")
        stf4 = consts.tile([P, n_chunks, R * (C + 1)], FP8, tag="stf4")
        bounds = [0]
        for s in SPANS:
            bounds.append(bounds[-1] + s)
        assert bounds[-1] == n_chunks
        for lo, hi in zip(bounds[:-1], bounds[1:]):
            sl = slice(lo * R * P, hi * R * P)
            nc.sync.dma_start(stft0[:, lo:hi, :, :], stft_d[0][:, sl])
            nc.sync.dma_start(stft1[:, lo:hi, :, :], stft_d[1][:, sl])
            nc.scalar.dma_start(stf4[:, lo:hi, :], stf4_d[:, lo:hi, :])

        # ---- PE warm-up: keep the HAM activity window busy while the first
        # DMA pieces stream in, so the real matmuls start at the warm clock.
        scratch = consts.tile([P, 512], FP8, tag="warm")
        nc.gpsimd.memset(scratch[:], 0)
        warm_ps = psum_w.tile([P, 512], F32)
        for _ in range(N_WARM):
            nc.tensor.matmul(
                warm_ps[:], scratch[:, :P], scratch[:],
                start=True, stop=True, skip_group_check=True,
            )

        # two PSUM accumulators (alternating per scatter matmul) so
        # consecutive accumulates never target the same bank back-to-back
        acc = [
            psum_acc.tile([L, C + 1], F32, tag="acc0", name="acc0"),
            psum_acc.tile([L, C + 1], F32, tag="acc1", name="acc1"),
        ]
        n_scatter = G * SC * R

        def scores(g):
            ps = psum_s.tile([P, SC * R, L], F32)
            for i in range(SC):
                chunk = g * SC + i
                for r in range(R):
                    nc.tensor.matmul(
                        ps[:, i * R + r, :], stft0[:, chunk, r, :],
                        qkt_t[:, 0, :], start=True, stop=False,
                    )
                    nc.tensor.matmul(
                        ps[:, i * R + r, :], stft1[:, chunk, r, :],
                        qkt_t[:, 1, :], start=False, stop=True,
                    )
            return ps

        def onehot_of(g, ps):
            sc_ap = ps[:]
            if with_qb:
                sc_sb = scb_pool.tile([P, SC * R, L], F32, tag="scb")
                nc.vector.tensor_tensor(
                    out=sc_sb[:], in0=ps[:],
                    in1=qb_t[:].unsqueeze(1).to_broadcast([P, SC * R, L]),
                    op=mybir.AluOpType.add,
                )
                sc_ap = sc_sb[:]
            # DVE: row max over l, then one-hot compare against the broadcast
            # max, both straight from PSUM fp32.
            rowmax = rm_pool.tile([P, SC * R], F32, tag="rmax")
            nc.vector.reduce_max(rowmax[:], sc_ap, axis=mybir.AxisListType.X)
            onehot = oh_pool.tile([P, SC * R, L], FP8, tag="oh")
            nc.vector.tensor_tensor(
                out=onehot[:], in0=sc_ap,
                in1=rowmax[:].unsqueeze(2).to_broadcast([P, SC * R, L]),
                op=mybir.AluOpType.is_ge,
            )
            return onehot

        def scatter(g, onehot):
            for i in range(SC):
                chunk = g * SC + i
                for r in range(R):
                    j = i * R + r
                    gidx = g * SC * R + j
                    nc.tensor.matmul(
                        acc[gidx % 2][:], onehot[:, j, :],
                        stf4[:, chunk, r * (C + 1) : (r + 1) * (C + 1)],
                        start=(gidx < 2), stop=(gidx >= n_scatter - 2),
                        skip_group_check=True,
                    )

        ps_g = scores(0)
        oh_g = None
        for g in range(G):
            ps_next = scores(g + 1) if g + 1 < G else None
            oh_g = onehot_of(g, ps_g)
            scatter(g, oh_g)
            ps_g = ps_next

        sums_tmp = consts.tile([L, C + 1], F32)
        nc.scalar.copy(sums_tmp[:], acc[0][:])
        sums_sb = consts.tile([L, C + 1], F32)
        nc.vector.tensor_tensor(
            out=sums_sb[:], in0=sums_tmp[:], in1=acc[1][:],
            op=mybir.AluOpType.add,
        )
        nc.sync.dma_start(out_d[:], sums_sb[:])

    if split:
        _split_sync_waits(nc)
    return nc


def _pack_shard(rows_f32: np.ndarray):
    """rows_f32: [rows, 256] f32 -> (stft [2,128,rows] fp8, stf4 [128,nc,1028] fp8)."""
    rows = rows_f32.shape[0]
    n_chunks = rows // (P * R)
    a = rows_f32.reshape(n_chunks, P, R, C)
    a8 = a.astype(NP_FP8)
    stf4 = np.ascontiguousarray(
        np.concatenate([a8, np.ones((n_chunks, P, R, 1), NP_FP8)], axis=-1)
        .transpose(1, 0, 2, 3)
    ).reshape(P, n_chunks, R * (C + 1))
    stft = np.ascontiguousarray(a8.transpose(3, 0, 2, 1)).reshape(2, P, rows)
    return stft, stf4


def _softmax(x, axis):
    m = np.max(x, axis=axis, keepdims=True)
    e = np.exp(x - m)
    return e / np.sum(e, axis=axis, keepdims=True)


def kernel(STFeature, centroidsTemp, qc_w, qc_b, nk_w, nk_b, nv_w, nv_b,
           al_w, al_b, mq_w, mq_b, mk_w, mk_b, mv_w, mv_b, mo_w, mo_b,
           bn_gamma, bn_beta, alpha, bias, ff1_w, ff1_b, ff2_w, ff2_b):
    global LAST_EXEC_TIME_NS, LAST_RESULTS
    f = np.float32
    STFeature = np.asarray(STFeature, f)
    centroidsTemp = np.asarray(centroidsTemp, f)

    # host-side prep (tiny): fold the node-key projection into the query side
    q_cent = centroidsTemp @ np.asarray(qc_w, f).T + np.asarray(qc_b, f)  # [B,L,C]
    qk = q_cent @ np.asarray(nk_w, f)                                     # [B,L,C]
    qb = q_cent @ np.asarray(nk_b, f)                                     # [B,L]
    with_qb = bool(np.any(qb != 0.0))

    in_maps = []
    flat = STFeature.reshape(B, T * N, C)
    for core in range(8):
        b, half = divmod(core, 2)
        stft, stf4 = _pack_shard(
            flat[b, half * ROWS_PER_CORE : (half + 1) * ROWS_PER_CORE]
        )
        m = {
            "stft": stft,
            "stf4": stf4,
            "qkt": np.ascontiguousarray(qk[b].T.reshape(2, P, L)).astype(NP_FP8),
        }
        if with_qb:
            m["qb_bc"] = np.ascontiguousarray(np.tile(qb[b][None, :], (P, 1)))
        in_maps.append(m)

    # the axon-proxied device occasionally reports a transient
    # NRT_EXEC_UNIT_UNRECOVERABLE; a fresh build+run attempt recovers it
    last_exc = None
    for attempt in range(3):
        try:
            nc = _build(N_CHUNKS, with_qb)
            res = run_bass_kernel_spmd(
                nc, in_maps, core_ids=list(range(8)), trace=bool(PROFILE)
            )
            break
        except Exception as e:
            last_exc = e
            import time as _time
            _time.sleep(15)
    else:
        raise last_exc
    LAST_EXEC_TIME_NS = res.exec_time_ns
    LAST_RESULTS = res

    sums = np.zeros((B, L, C), f)
    counts = np.zeros((B, L), f)
    for b in range(B):
        p0 = res.results[2 * b]["out_sums"]
        p1 = res.results[2 * b + 1]["out_sums"]
        sums[b] = p0[:, :C] + p1[:, :C]
        counts[b] = p0[:, C] + p1[:, C]

    # tiny epilogue on host, fp32 (mirrors the reference math)
    sums_v = sums @ np.asarray(nv_w, f).T + counts[..., None] * np.asarray(nv_b, f)
    cluster = sums_v / (counts**2 + 1.0)[..., None]
    cent = centroidsTemp + cluster @ np.asarray(al_w, f).T + np.asarray(al_b, f)

    D = cent.shape[-1]
    hd = D // N_HEADS
    q = (cent @ np.asarray(mq_w, f).T + np.asarray(mq_b, f)).reshape(B, L, N_HEADS, hd)
    k = (cent @ np.asarray(mk_w, f).T + np.asarray(mk_b, f)).reshape(B, L, N_HEADS, hd)
    v = (cent @ np.asarray(mv_w, f).T + np.asarray(mv_b, f)).reshape(B, L, N_HEADS, hd)
    logits = np.einsum("bqhd,bkhd->bhqk", q, k) / np.sqrt(f(hd))
    attn = _softmax(logits, axis=-1)
    attn_out = np.einsum("bhqk,bkhd->bqhd", attn, v).reshape(B, L, D)
    attn_out = attn_out @ np.asarray(mo_w, f).T + np.asarray(mo_b, f)

    z2 = cent + attn_out
    mean = z2.mean(axis=(0, 1))
    var = ((z2 - mean) ** 2).mean(axis=(0, 1))
    zn = (z2 - mean) / np.sqrt(var + f(BN_EPS))
    zn = np.asarray(bn_gamma, f) * zn + np.asarray(bn_beta, f)
    zn = np.asarray(alpha, f) * zn + np.asarray(bias, f)

    h = np.maximum(zn @ np.asarray(ff1_w, f).T + np.asarray(ff1_b, f), 0.0)
    out = h @ np.asarray(ff2_w, f).T + np.asarray(ff2_b, f)
    return out.astype(np.float32)
